# revision 1
# baseline (speedup 1.0000x reference)
"""Trainium2 Bass kernel for nn_FIoUCriterion (pairwise-overlap IoU-style loss).

Strategy (8 NeuronCores, data-parallel over batch):
  - Host: cast masks (32,64,128,128) f32 -> bf16, reshape per-core shard to
    (256, 16384) rows=(4 local batches x 64 nodes); derive the static per-pair
    beta / weight matrices from `nodes` (pure index logic).
  - Device (per core): batched DMA-xbar-transpose loads pixel-major tiles,
    one fused DVE op y = max(x,-1)+1 (= 2*m), PE grams with 2-batch stacking:
    for each 128-pixel chunk and each batch pair, matmul(lhsT=tile, rhs=tile)
    accumulates y@y.T into PSUM, plus a N=1 matmul against a ones column
    accumulates s = sum_k y.  Epilogue: r = 1/s, C = gram*r_i,
    cr_contrib = max(C, C^T) (valid since gram>=0), accumulate over local
    pairs, AllGather+local-sum of the (64,64) partials across 8 cores, then
    loss = sum(|beta - cr_sum/64| * wgt2) with symmetrized normalized weights
    wgt2 = (wgt + wgt^T) / (2*sum(wgt)).
  - Scale bookkeeping: y = 2m  => gram_psum = 4*gram, s_psum = 2*s,
    max(C,C^T) = 2*cr_b; sum over 32 batches then *1/64 gives mean cr.
"""

import numpy as np
import ml_dtypes

N_CORES = 8
B, N, H, W = 32, 64, 128, 128
HW = H * W
B_LOC = B // N_CORES          # 4 batches per core
ROWS = B_LOC * N              # 256
N_PAIRS = B_LOC // 2          # 2 stacked pairs per core
N_CHUNK = HW // 128           # 128 pixel chunks
N_SEPARATE = 7
N_FLEXIBLE = 2

_cached = {}


def _build_bass(with_collective: bool = True, bench_loop: int | None = None,
                phase: str = "full", variant: int = 1,
                fold_mode: str = "pe", cc_mode: str = "ag", dual_ring: bool = False,
                calls_by_pair=None, stream_bufs: int = 6):
    import contextlib
    import concourse.bacc as bacc
    import concourse.mybir as mybir
    import concourse.tile as tile

    f32 = mybir.dt.float32
    bf16 = mybir.dt.bfloat16
    Alu = mybir.AluOpType

    nc = bacc.Bacc("TRN2", target_bir_lowering=False, debug=False, num_devices=N_CORES)
    xb = nc.dram_tensor("xb", [ROWS, HW], bf16, kind="ExternalInput")
    beta_d = nc.dram_tensor("beta", [N, N], f32, kind="ExternalInput")
    wgt2_d = nc.dram_tensor("wgt2", [N, N], f32, kind="ExternalInput")
    loss_d = nc.dram_tensor("loss", [1, 1], f32, kind="ExternalOutput")

    def emit(tc, const, stream, ep, gpsum, tpsum, dram):
        # --- constants ---
        ones_bf = const.tile([128, 1], bf16)
        nc.vector.memset(ones_bf[:], 1.0)
        ones_f32 = const.tile([N, 1], f32)
        nc.vector.memset(ones_f32[:], 1.0)
        ident = const.tile([128, 128], f32)
        from concourse import masks as masks_lib
        masks_lib.make_identity(nc, ident[:])
        beta_t = const.tile([N, N], f32)
        nc.sync.dma_start(beta_t[:], beta_d[:])
        wgt2_t = const.tile([N, N], f32)
        nc.sync.dma_start(wgt2_t[:], wgt2_d[:])

        # --- PSUM accumulators (persist across the stream) ---
        g_acc = [gpsum.tile([128, 128], f32, name=f"g_acc{p}") for p in range(N_PAIRS)]
        s_acc = [gpsum.tile([128, 1], f32, name=f"s_acc{p}") for p in range(N_PAIRS)]

        bench_cm = (tc.For_i(0, bench_loop, 1, hint_engines=(mybir.EngineType.PE,))
                    if bench_loop else contextlib.nullcontext())
        bench_cm.__enter__()

        acc = ep.tile([128, 128], f32)

        def pair_epilogue(p):
            r = ep.tile([128, 1], f32, name=f"r{p}")
            nc.vector.reciprocal(r[:], s_acc[p][:])
            C = ep.tile([128, 128], f32, name=f"C{p}")
            nc.vector.tensor_scalar_mul(C[:], g_acc[p][:], r[:])
            CT = tpsum.tile([128, 128], f32, name=f"CT{p}", tag="CT", bufs=2)
            nc.tensor.transpose(CT[:], C[:], ident[:])
            if p == 0:
                nc.vector.tensor_max(acc[:], C[:], CT[:])
            else:
                mx = ep.tile([128, 128], f32, name=f"mx{p}")
                nc.vector.tensor_max(mx[:], C[:], CT[:])
                nc.vector.tensor_add(acc[:], acc[:], mx[:])

        if phase == "dmaplain":
            for half in range(2):
                for q in range(4):
                    tp = stream.tile([128, 4096], bf16, name="tp")
                    nc.sync.dma_start(tp[:], xb[half * 128:(half + 1) * 128,
                                                q * 4096:(q + 1) * 4096])
        elif variant == 1:
            # --- streaming: batched xbar transposes, X chunks per DMA ---
            X = 16                       # 128-pixel chunks per transpose call
            n_big = N_CHUNK // X         # 8 mega-tiles
            for c2 in range(n_big if phase != "noop" else 0):
                t = stream.tile([128, X * ROWS], bf16, name="t")
                c0 = c2 * X * 128
                # out[p, i, f] = xb[f, c0 + i*128 + p]  (verified on HW)
                nc.sync.dma_start(t[:].rearrange("p (x f) -> p x f", x=X),
                                  xb[0:ROWS, c0:c0 + X * 128], transpose=True)
                # y = max(x, -1) + 1   (= 2*m), in place, bf16 4x mode
                if phase != "dma":
                    nc.vector.tensor_scalar(t[:], t[:], -1.0, 1.0, Alu.max, Alu.add)
                if phase in ("dma", "dma_relu"):
                    continue
                for k in range(X):       # chunk within the mega tile
                    first = (c2 == 0 and k == 0)
                    last = (c2 == n_big - 1 and k == X - 1)
                    for p in range(N_PAIRS):
                        sl = t[:, k * ROWS + p * 128: k * ROWS + (p + 1) * 128]
                        nc.tensor.matmul(g_acc[p][:], lhsT=sl, rhs=sl,
                                         start=first, stop=last)
                        nc.tensor.matmul(s_acc[p][:], lhsT=sl, rhs=ones_bf[:],
                                         start=first, stop=last)
        else:
            # --- v2: pair-split streaming; pair0's epilogue overlaps pair1's
            # stream.  Tapered call sizes shrink the serial tail. ---
            CALLS_BY_PAIR = calls_by_pair or [
                [8, 16, 40, 64],          # pair0: fill the pipeline fast
                [48, 40, 24, 8, 8],       # pair1: shrink the serial tail
            ]
            dma_i = 0
            for p in range(N_PAIRS if phase != "noop" else 0):
                CALLS = CALLS_BY_PAIR[p]
                assert sum(CALLS) == N_CHUNK
                row0 = p * 128
                c0 = 0
                for ci, Xc in enumerate(CALLS):
                    # dedicated buffer per call (whole shard = 64KB/partition):
                    # no slot-reuse WAW stalls, DMA queue can run arbitrarily deep
                    t = stream.tile([128, Xc * 128], bf16, name="t",
                                    tag=f"t{p}_{ci}", bufs=1)
                    tv = t[:, 0:Xc * 128]
                    # alternate the two HWDGE rings (SP / ACT)
                    eng = nc.sync if (dma_i % 2 == 0 or not dual_ring) else nc.scalar
                    dma_i += 1
                    eng.dma_start(
                        tv.rearrange("q (x f) -> q x f", x=Xc),
                        xb[row0:row0 + 128, c0 * 128:(c0 + Xc) * 128],
                        transpose=True)
                    if phase != "dma":
                        # split relu so the first chunks' matmuls can start
                        # while the rest of the call is still in the DVE
                        h = (Xc // 2) * 128 if Xc > 8 else Xc * 128
                        nc.vector.tensor_scalar(t[:, 0:h], t[:, 0:h],
                                                -1.0, 1.0, Alu.max, Alu.add)
                        if h < Xc * 128:
                            nc.vector.tensor_scalar(t[:, h:Xc * 128], t[:, h:Xc * 128],
                                                    -1.0, 1.0, Alu.max, Alu.add)
                    if phase not in ("dma", "dma_relu"):
                        for k in range(Xc):
                            first = (ci == 0 and k == 0)
                            last = (ci == len(CALLS) - 1 and k == Xc - 1)
                            sl = t[:, k * 128:(k + 1) * 128]
                            nc.tensor.matmul(g_acc[p][:], lhsT=sl, rhs=sl,
                                             start=first, stop=last)
                            nc.tensor.matmul(s_acc[p][:], lhsT=sl, rhs=ones_bf[:],
                                             start=first, stop=last)
                    c0 += Xc
                if phase == "full":
                    pair_epilogue(p)

        if phase in ("noop", "dma", "dmaplain", "dma_relu", "stream"):
            lout0 = ep.tile([1, 1], f32)
            nc.vector.memset(lout0[:], 0.0)
            nc.sync.dma_start(loss_d[:], lout0[:])
            bench_cm.__exit__(None, None, None)
            return

        if variant == 1:
            for p in range(N_PAIRS):
                pair_epilogue(p)

        # fold the two stacked 64-blocks: local cr partial (64,64).
        if variant == 1 or fold_mode == "dma":
            # engines can't move data across partitions; small SBUF->SBUF DMA
            blk1 = ep.tile([N, N], f32)
            nc.sync.dma_start(blk1[:], acc[N:128, N:128])
            crl = ep.tile([N, N], f32)
            nc.vector.tensor_add(crl[:], acc[0:N, 0:N], blk1[:])
        else:
            # PE transpose moves block1 down to partitions 0:64; the block is
            # symmetric so the transpose is a no-op on values.
            blk1p = tpsum.tile([N, N], f32, name="blk1p")
            nc.tensor.transpose(blk1p[:], acc[N:128, N:128], ident[N:128, N:128])
            crl = ep.tile([N, N], f32)
            nc.vector.tensor_add(crl[:], acc[0:N, 0:N], blk1p[:])

        # --- combine partials across the 8 cores ---
        # AllGather (floor ~4.6us on 8 cores) + local sum beats AllReduce
        # (floor ~9.7us) at this size.
        if with_collective and cc_mode == "ar":
            cc_in0 = dram.tile([N, N], f32)
            cc_out0 = dram.tile([N, N], f32, addr_space="Shared")
            nc.sync.dma_start(cc_in0[:], crl[:])
            nc.gpsimd.collective_compute(
                "AllReduce", Alu.add,
                replica_groups=[list(range(N_CORES))],
                ins=[cc_in0.opt()], outs=[cc_out0.opt()],
            )
            crs = ep.tile([N, N], f32, name="crs_ar")
            nc.sync.dma_start(crs[:], cc_out0[:])
        elif with_collective:
            cc_in = dram.tile([N, N], f32)
            cc_ag = dram.tile([N_CORES * N, N], f32, addr_space="Shared")
            nc.sync.dma_start(cc_in[:], crl[:])
            nc.gpsimd.collective_compute(
                "AllGather", Alu.bypass,
                replica_groups=[list(range(N_CORES))],
                ins=[cc_in.opt()], outs=[cc_ag.opt()],
            )
            # gather back as (64, r, 64): S[i, r, j] = AG[r*64+i, j]
            sg = ep.tile([N, N_CORES * N], f32)
            nc.sync.dma_start(
                sg[:].rearrange("i (r j) -> i r j", r=N_CORES),
                cc_ag[:].rearrange("(r i) j -> i r j", r=N_CORES))
            crs = ep.tile([N, N], f32)
            # reduce over r: view free dim as (j outer, r inner) and reduce X
            nc.vector.tensor_reduce(
                crs[:], sg[:].rearrange("i (r j) -> i j r", r=N_CORES),
                mybir.AxisListType.X, Alu.add)
        else:
            crs = crl

        # --- final reduction ---
        u = ep.tile([N, N], f32)
        # u = (crs * 1/64) - beta
        nc.vector.scalar_tensor_tensor(u[:], crs[:], 1.0 / 64.0, beta_t[:],
                                       Alu.mult, Alu.subtract)
        v = ep.tile([N, N], f32)
        nc.vector.tensor_mul(v[:], u[:], wgt2_t[:])
        vr = ep.tile([N, 1], f32)
        nc.vector.tensor_reduce(vr[:], v[:], mybir.AxisListType.X, Alu.add,
                                apply_absolute_value=True)
        lps = tpsum.tile([1, 1], f32)
        nc.tensor.matmul(lps[:], lhsT=vr[:], rhs=ones_f32[:], start=True, stop=True)
        lout = ep.tile([1, 1], f32)
        nc.vector.tensor_copy(lout[:], lps[:])
        nc.sync.dma_start(loss_d[:], lout[:])

        bench_cm.__exit__(None, None, None)

    with tile.TileContext(nc) as tc:
        with tc.tile_pool(name="const", bufs=1) as const, \
             tc.tile_pool(name="stream", bufs=stream_bufs) as stream, \
             tc.tile_pool(name="ep", bufs=1) as ep, \
             tc.tile_pool(name="gpsum", bufs=1, space="PSUM") as gpsum, \
             tc.tile_pool(name="tpsum", bufs=1, space="PSUM") as tpsum, \
             tc.tile_pool(name="dram", bufs=1, space="DRAM") as dram:
            emit(tc, const, stream, ep, gpsum, tpsum, dram)

    nc.compile()
    return nc


def _host_prep(masks: np.ndarray, nodes: np.ndarray):
    xb = masks.reshape(B, N, HW).astype(ml_dtypes.bfloat16)
    shards = [np.ascontiguousarray(xb[c * B_LOC:(c + 1) * B_LOC].reshape(ROWS, HW))
              for c in range(N_CORES)]

    t = np.where(nodes < N_SEPARATE, 0, np.where(nodes < N_SEPARATE + N_FLEXIBLE, 1, 2))
    ti, tj = t[:, None], t[None, :]
    has_f = (ti == 1) | (tj == 1)
    has_a = (ti == 2) | (tj == 2)
    include = ~(has_f & ~has_a)
    beta = ((ti == 2) ^ (tj == 2)).astype(np.float32)
    triu = np.triu(np.ones((N, N), bool), k=1)
    wgt = (include & triu).astype(np.float64)
    wgt2 = ((wgt + wgt.T) / (2.0 * wgt.sum())).astype(np.float32)
    return shards, beta, wgt2


def kernel(masks: np.ndarray, nodes: np.ndarray) -> np.ndarray:
    from concourse.bass_utils import run_bass_kernel_spmd

    masks = np.asarray(masks, dtype=np.float32)
    nodes = np.asarray(nodes)
    shards, beta, wgt2 = _host_prep(masks, nodes)

    if "nc" not in _cached:
        _cached["nc"] = _build_bass(variant=2)
    nc = _cached["nc"]

    in_maps = [{"xb": shards[c], "beta": beta, "wgt2": wgt2} for c in range(N_CORES)]
    try:
        res = run_bass_kernel_spmd(nc, in_maps, core_ids=list(range(N_CORES)))
    except Exception:
        res = run_bass_kernel_spmd(nc, in_maps, core_ids=list(range(N_CORES)))
    loss = np.float32(res.results[0]["loss"][0, 0])
    return np.asarray(loss, dtype=np.float32).reshape(())



# revision 26
# speedup vs baseline: 1.3664x; 1.3664x over previous
"""Trainium2 Bass kernel for nn_FIoUCriterion (pairwise-overlap IoU-style loss).

Strategy (8 NeuronCores, data-parallel over batch; 4 batches/core):
  - Host prep is layout/quantization only: each core's shard (4 batches x 64
    nodes x 16384 pixels) is pre-transposed to pixel-major so the device needs
    NO xbar-transpose DMA (plain contiguous loads run at the ~358 GB/s HBM
    roofline vs ~261 GB/s for DMA-transpose).  A 129th "ones column" is
    interleaved after every 128-column chunk so a single N=129 matmul per
    chunk accumulates BOTH the Gram block and the per-row mass sums s into
    one PSUM tile (the baseline used a second N=1 matmul per chunk, which
    doubled the PE LDWEIGHTS traffic).
  - mode="bf16": raw x values are shipped (bf16); the device applies the
    fused DVE op y = max(x,-1)+1 (= 2*m) in 4x mode; ones-columns are stored
    as 0 so the same op maps them to exactly 1.0.
  - mode="fp8dr": the host quantizes y = max(x,-1)+1 directly to fp8e4m3
    (fp8 range is wasted on raw x: everything below -1 collapses to y=0, so
    encoding y is the information-preserving quantization).  The device runs
    fp8 matmuls in DoubleRow perf mode: 256-pixel contraction per matmul
    (2 pixels per PE cell), halving both DMA bytes and matmul count.
    Verified on HW: DoubleRow pairs lhsT/rhs k-tiles same-o with layout
    [partition, o, col], o-step % 16 == 0.
  - Per batch-pair p (2 batches stacked on 128 partitions): PSUM acc
    [128,129] accumulates over pixel chunks; epilogue r = 1/s,
    C = gram * r_i, cr_contrib = max(C, C^T) (valid since gram >= 0),
    accumulated over local pairs; AllGather + local-sum of the (64,64)
    partials across 8 cores; loss = sum(|beta - crs/64| * wgt2) with
    symmetrized normalized weights wgt2 = (wgt + wgt^T) / (2*sum(wgt)).
  - Scheduling details that matter: pair p's epilogue is emitted after pair
    p+1's first call so its PE transpose never blocks the (FIFO) PE queue
    while waiting on DVE results; the last two DMA calls use a per-chunk
    padded layout so the final call is only 4 chunks (short serial MM tail
    after the last DMA lands); the loss write rides the scalar HWDGE ring so
    it never FIFO-blocks stream DMAs on the sync ring.
  - Scale bookkeeping: y = 2m => gram = 4*gram_m, s = 2*s_m,
    max(C,C^T) = 2*cr_b; summed over 32 batches, /64 gives mean cr.
"""

import numpy as np
import ml_dtypes

N_CORES = 8
B, N, H, W = 32, 64, 128, 128
HW = H * W
B_LOC = B // N_CORES          # 4 batches per core
ROWS = B_LOC * N              # 256
N_PAIRS = B_LOC // 2          # 2 stacked pairs per core
N_SEPARATE = 7
N_FLEXIBLE = 2

MODE = "fp8dr"                # "bf16" | "fp8" | "fp8dr"

# chunks per pair: bf16/fp8 contract 128 pixels per matmul, fp8dr 256
def _n_chunk(mode):
    return HW // 256 if mode == "fp8dr" else HW // 128

# tapered DMA call sizes (in chunks) per pair: pair0 fills the pipeline fast,
# pair1 shrinks the serial tail.  fp8dr o-block calls must be multiples of 16
# chunks (o-dim AP step X*129 bytes must be divisible by 16); NEGATIVE entries
# mark per-chunk-padded calls ([chunk][o][144] layout, o-step 144B) which
# allow any size -- used to make the final call tiny so the serial MM tail
# after the last DMA is short.
def _calls(mode):
    if mode == "fp8dr":
        return [[16, 48], [16, 16, 16, -12, -4]]
    return [[8, 24, 96], [48, 40, 24, 8, 8]]


def _call_cols(mode, Xc):
    # free-dim columns occupied by one call's tile
    if mode != "fp8dr":
        return Xc * 129
    return -Xc * 288 if Xc < 0 else Xc * 258

_cached = {}


def _build_bass(with_collective: bool = True, bench_loop: int | None = None,
                phase: str = "full", mode: str = MODE,
                calls_by_pair=None, stream_bufs: int = 6, dual_ring: bool = False):
    import contextlib
    import concourse.bacc as bacc
    import concourse.mybir as mybir
    import concourse.tile as tile

    f32 = mybir.dt.float32
    bf16 = mybir.dt.bfloat16
    f8 = mybir.dt.float8e4
    Alu = mybir.AluOpType

    n_chunk = _n_chunk(mode)
    in_dt = f8 if mode.startswith("fp8") else bf16
    o_fac = 2 if mode == "fp8dr" else 1
    calls0 = calls_by_pair or _calls(mode)
    total_cols = sum(_call_cols(mode, X) for pc in calls0 for X in pc)

    nc = bacc.Bacc("TRN2", target_bir_lowering=False, debug=False, num_devices=N_CORES)
    xb = nc.dram_tensor("xb", [128, total_cols], in_dt, kind="ExternalInput")
    beta_d = nc.dram_tensor("beta", [N, N], f32, kind="ExternalInput")
    wgt2_d = nc.dram_tensor("wgt2", [N, N], f32, kind="ExternalInput")
    loss_d = nc.dram_tensor("loss", [1, 1], f32, kind="ExternalOutput")

    def emit(tc, const, stream, ep, gpsum, tpsum, dram):
        # --- constants ---
        ones_f32 = const.tile([N, 1], f32)
        nc.vector.memset(ones_f32[:], 1.0)
        ident = const.tile([128, 128], f32)
        from concourse import masks as masks_lib
        masks_lib.make_identity(nc, ident[:])
        beta_t = const.tile([N, N], f32)
        nc.sync.dma_start(beta_t[:], beta_d[:])
        wgt2_t = const.tile([N, N], f32)
        nc.sync.dma_start(wgt2_t[:], wgt2_d[:])

        # --- PSUM accumulators: [gram | s] per pair ---
        g_acc = [gpsum.tile([128, 129], f32, name=f"g_acc{p}") for p in range(N_PAIRS)]

        bench_cm = (tc.For_i(0, bench_loop, 1, hint_engines=(mybir.EngineType.PE,))
                    if bench_loop else contextlib.nullcontext())
        bench_cm.__enter__()

        acc = ep.tile([128, 128], f32)

        def pair_epilogue(p):
            r = ep.tile([128, 1], f32, name=f"r{p}")
            nc.vector.reciprocal(r[:], g_acc[p][:, 128:129])
            C = ep.tile([128, 128], f32, name=f"C{p}")
            nc.vector.tensor_scalar_mul(C[:], g_acc[p][:, 0:128], r[:])
            CT = tpsum.tile([128, 128], f32, name=f"CT{p}", tag="CT", bufs=2)
            nc.tensor.transpose(CT[:], C[:], ident[:])
            if p == 0:
                nc.vector.tensor_max(acc[:], C[:], CT[:])
            else:
                mx = ep.tile([128, 128], f32, name=f"mx{p}")
                nc.vector.tensor_max(mx[:], C[:], CT[:])
                nc.vector.tensor_add(acc[:], acc[:], mx[:])

        CALLS_BY_PAIR = calls_by_pair or _calls(mode)
        # deferred epilogues: pair p's epilogue is emitted after pair p+1's
        # first call so its PE transpose never stalls the PE FIFO (its DVE
        # inputs are long ready by the time the PE drains the next call's
        # matmuls)
        pending_epilogue = []
        doff = 0
        for p in range(N_PAIRS if phase != "noop" else 0):
            CALLS = CALLS_BY_PAIR[p]
            assert sum(abs(X) for X in CALLS) == n_chunk
            c0 = 0
            for ci, Xc in enumerate(CALLS):
                padded = Xc < 0
                Xa = abs(Xc)
                cols = _call_cols(mode, Xc)
                if ci == 1 and pending_epilogue and phase in ("full", "epi"):
                    pair_epilogue(pending_epilogue.pop())
                # dedicated buffer per call: no slot-reuse WAW stalls, DMA
                # queue can run arbitrarily deep
                t = stream.tile([128, cols], in_dt, name="t",
                                tag=f"t{p}_{ci}", bufs=1)
                if phase == "pe":
                    # no DMA: time the matmul stream; tiny memset allocates
                    # the tile without meaningful DVE cost
                    nc.vector.memset(t[:, 0:16], 1.0)
                elif mode == "fp8dr":
                    # DRAM is laid out call-major ([pair][call][o][k][129])
                    # so every call is one flat contiguous read per partition
                    eng = nc.scalar if (dual_ring and (p * 8 + ci) % 2) else nc.sync
                    eng.dma_start(t[:], xb[:, doff:doff + cols])
                else:
                    nc.sync.dma_start(t[:], xb[:, doff:doff + cols])
                    if phase != "dma" and mode == "bf16":
                        # y = max(x, -1) + 1 (= 2*m) in place, bf16 4x mode;
                        # split so the first chunks' matmuls can start while
                        # the rest of the call is still in the DVE
                        tot = Xc * 129
                        h = (tot // 2) & ~1 if Xc > 8 else tot
                        nc.vector.tensor_scalar(t[:, 0:h], t[:, 0:h],
                                                -1.0, 1.0, Alu.max, Alu.add)
                        if h < tot:
                            nc.vector.tensor_scalar(t[:, h:tot], t[:, h:tot],
                                                    -1.0, 1.0, Alu.max, Alu.add)
                if phase in ("dma", "dma_relu"):
                    c0 += Xa
                    doff += cols
                    continue
                if mode == "fp8dr":
                    t3 = None if padded else t[:].rearrange("q (o f) -> q o f", o=2)
                    for k in range(Xa):
                        first = (ci == 0 and k == 0)
                        last = (ci == len(CALLS) - 1 and k == Xa - 1)
                        if padded:
                            v = t[:, k * 288:(k + 1) * 288].rearrange(
                                "q (o j) -> q o j", o=2)
                            lhsT, rhs = v[:, :, 0:128], v[:, :, 0:129]
                        else:
                            lhsT = t3[:, :, k * 129:k * 129 + 128]
                            rhs = t3[:, :, k * 129:(k + 1) * 129]
                        nc.tensor.matmul(
                            g_acc[p][:], lhsT=lhsT, rhs=rhs,
                            start=first, stop=last,
                            perf_mode=mybir.MatmulPerfMode.DoubleRow)
                else:
                    for k in range(Xc):
                        first = (ci == 0 and k == 0)
                        last = (ci == len(CALLS) - 1 and k == Xc - 1)
                        nc.tensor.matmul(
                            g_acc[p][:], lhsT=t[:, k * 129:k * 129 + 128],
                            rhs=t[:, k * 129:(k + 1) * 129],
                            start=first, stop=last)
                c0 += Xa
                doff += cols
            if phase in ("full", "epi"):
                if p < N_PAIRS - 1:
                    pending_epilogue.append(p)
                else:
                    pair_epilogue(p)

        if phase in ("noop", "dma", "dma_relu", "stream", "pe"):
            bench_cm.__exit__(None, None, None)
            lout0 = ep.tile([1, 1], f32)
            nc.vector.memset(lout0[:], 0.0)
            nc.sync.dma_start(loss_d[:], lout0[:])
            return

        # fold the two stacked 64-blocks: local cr partial (64,64).  PE
        # transpose moves block1 down to partitions 0:64; the block is
        # symmetric so the transpose is a no-op on values.
        blk1p = tpsum.tile([N, N], f32, name="blk1p")
        nc.tensor.transpose(blk1p[:], acc[N:128, N:128], ident[N:128, N:128])
        crl = ep.tile([N, N], f32)
        nc.vector.tensor_add(crl[:], acc[0:N, 0:N], blk1p[:])

        if phase == "epi":
            bench_cm.__exit__(None, None, None)
            lout0 = ep.tile([1, 1], f32)
            nc.vector.memset(lout0[:], 0.0)
            nc.sync.dma_start(loss_d[:], lout0[:])
            return

        # --- combine partials across the 8 cores ---
        # AllGather (floor ~4.6us on 8 cores) + local sum beats AllReduce
        # (floor ~9.7us) at this size.
        if with_collective:
            cc_in = dram.tile([N, N], f32)
            cc_ag = dram.tile([N_CORES * N, N], f32, addr_space="Shared")
            nc.sync.dma_start(cc_in[:], crl[:])
            nc.gpsimd.collective_compute(
                "AllGather", Alu.bypass,
                replica_groups=[list(range(N_CORES))],
                ins=[cc_in.opt()], outs=[cc_ag.opt()],
            )
            # gather back as (64, r, 64): S[i, r, j] = AG[r*64+i, j]
            sg = ep.tile([N, N_CORES * N], f32)
            nc.sync.dma_start(
                sg[:].rearrange("i (r j) -> i r j", r=N_CORES),
                cc_ag[:].rearrange("(r i) j -> i r j", r=N_CORES))
            crs = ep.tile([N, N], f32)
            # reduce over r: view free dim as (j outer, r inner) and reduce X
            nc.vector.tensor_reduce(
                crs[:], sg[:].rearrange("i (r j) -> i j r", r=N_CORES),
                mybir.AxisListType.X, Alu.add)
        else:
            crs = crl

        # --- final reduction ---
        u = ep.tile([N, N], f32)
        # u = (crs * 1/64) - beta
        nc.vector.scalar_tensor_tensor(u[:], crs[:], 1.0 / 64.0, beta_t[:],
                                       Alu.mult, Alu.subtract)
        v = ep.tile([N, N], f32)
        nc.vector.tensor_mul(v[:], u[:], wgt2_t[:])
        vr = ep.tile([N, 1], f32)
        nc.vector.tensor_reduce(vr[:], v[:], mybir.AxisListType.X, Alu.add,
                                apply_absolute_value=True)
        lps = tpsum.tile([1, 1], f32)
        nc.tensor.matmul(lps[:], lhsT=vr[:], rhs=ones_f32[:], start=True, stop=True)
        lout = ep.tile([1, 1], f32)
        nc.vector.tensor_copy(lout[:], lps[:])
        # scalar HWDGE ring: keeps the loss write off the sync ring so the
        # next iteration's stream DMAs aren't FIFO-blocked behind it
        nc.scalar.dma_start(loss_d[:], lout[:])

        bench_cm.__exit__(None, None, None)

    with tile.TileContext(nc) as tc:
        with tc.tile_pool(name="const", bufs=1) as const, \
             tc.tile_pool(name="stream", bufs=stream_bufs) as stream, \
             tc.tile_pool(name="ep", bufs=1) as ep, \
             tc.tile_pool(name="gpsum", bufs=1, space="PSUM") as gpsum, \
             tc.tile_pool(name="tpsum", bufs=1, space="PSUM") as tpsum, \
             tc.tile_pool(name="dram", bufs=1, space="DRAM") as dram:
            emit(tc, const, stream, ep, gpsum, tpsum, dram)

    nc.compile()
    return nc


def _host_prep(masks: np.ndarray, nodes: np.ndarray, mode: str = MODE):
    n_chunk = _n_chunk(mode)
    xb = masks.reshape(B, N, HW)
    shards = []
    for c in range(N_CORES):
        sh = xb[c * B_LOC:(c + 1) * B_LOC].reshape(ROWS, HW)
        if mode == "fp8dr":
            # y = max(x,-1)+1 quantized to fp8e4m3; call-major layout
            # [pix 128][pair][call][o 2][chunk-in-call][col 129], ones at 128
            y = np.maximum(sh, -1.0) + 1.0
            ys = y.reshape(N_PAIRS, 128, n_chunk, 2, 128)  # [p,row,c,o,pix]
            base = np.empty((128, N_PAIRS, 2, n_chunk, 129), np.float32)
            base[..., :128] = ys.transpose(4, 0, 3, 2, 1)
            base[..., 128] = 1.0
            parts = []
            for p in range(N_PAIRS):
                c0 = 0
                for Xc in _calls(mode)[p]:
                    if Xc > 0:      # o-block layout [o][chunk][129]
                        parts.append(base[:, p, :, c0:c0 + Xc, :].reshape(128, -1))
                        c0 += Xc
                    else:           # padded per-chunk layout [chunk][o][144]
                        X = -Xc
                        blk = np.zeros((128, X, 2, 144), np.float32)
                        blk[..., :129] = base[:, p, :, c0:c0 + X, :].transpose(
                            0, 2, 1, 3)
                        parts.append(blk.reshape(128, -1))
                        c0 += X
            shards.append(np.ascontiguousarray(
                np.concatenate(parts, axis=1).astype(ml_dtypes.float8_e4m3)))
        elif mode == "fp8":
            # host-quantized y (DVE has no fast fp8 elementwise path)
            y = np.maximum(sh, -1.0) + 1.0
            ys = y.reshape(N_PAIRS, 128, n_chunk, 128)     # [p,row,c,pix]
            arr = np.empty((128, N_PAIRS, n_chunk, 129), np.float32)
            arr[..., :128] = ys.transpose(3, 0, 2, 1)
            arr[..., 128] = 1.0
            shards.append(np.ascontiguousarray(
                arr.reshape(128, -1).astype(ml_dtypes.float8_e4m3)))
        else:
            # raw x in bf16; ones col stored as 0 -> device relu makes 1.0
            xs = sh.reshape(N_PAIRS, 128, n_chunk, 128)    # [p,row,c,pix]
            arr = np.zeros((128, N_PAIRS, n_chunk, 129), np.float32)
            arr[..., :128] = xs.transpose(3, 0, 2, 1)
            shards.append(np.ascontiguousarray(
                arr.reshape(128, -1).astype(ml_dtypes.bfloat16)))

    t = np.where(nodes < N_SEPARATE, 0, np.where(nodes < N_SEPARATE + N_FLEXIBLE, 1, 2))
    ti, tj = t[:, None], t[None, :]
    has_f = (ti == 1) | (tj == 1)
    has_a = (ti == 2) | (tj == 2)
    include = ~(has_f & ~has_a)
    beta = ((ti == 2) ^ (tj == 2)).astype(np.float32)
    triu = np.triu(np.ones((N, N), bool), k=1)
    wgt = (include & triu).astype(np.float64)
    wgt2 = ((wgt + wgt.T) / (2.0 * wgt.sum())).astype(np.float32)
    return shards, beta, wgt2


def kernel(masks: np.ndarray, nodes: np.ndarray) -> np.ndarray:
    from concourse.bass_utils import run_bass_kernel_spmd

    masks = np.asarray(masks, dtype=np.float32)
    nodes = np.asarray(nodes)
    shards, beta, wgt2 = _host_prep(masks, nodes)

    if "nc" not in _cached:
        _cached["nc"] = _build_bass()
    nc = _cached["nc"]

    in_maps = [{"xb": shards[c], "beta": beta, "wgt2": wgt2} for c in range(N_CORES)]
    try:
        res = run_bass_kernel_spmd(nc, in_maps, core_ids=list(range(N_CORES)))
    except Exception:
        res = run_bass_kernel_spmd(nc, in_maps, core_ids=list(range(N_CORES)))
    loss = np.float32(res.results[0]["loss"][0, 0])
    return np.asarray(loss, dtype=np.float32).reshape(())
